# revision 13
# baseline (speedup 1.0000x reference)
"""BiLSTM-CRF loss (negative log-likelihood) Trainium2 Bass kernel.

Strategy: pure data parallel over 8 NeuronCores (128 sentences each).
On each core a single merged 68-state scaled-probability forward recursion
computes BOTH the log-partition function Z and the gold path score:

  state rows:  0..31  u-real   (forward probs, exp domain)
               32..63 g-real   (gold path product chain, one-hot gated)
               64     u-cap    (captures terminal sum at t == len)
               65     u-hold   (carries captured value to the end)
               66     g-cap
               67     g-hold
  The per-step matmul also emits rows 96/97 = column sums of the u/g
  blocks, used for periodic (every 8 steps) per-sentence renormalization;
  both chains are scaled per block and the scale logs are accumulated, so
  NLL = Z - gold = [ln(capsum_u) + sum ln c_u] - [ln(capsum_g) + sum ln c_g].

Per time step: one PE transpose (feature column -> [68,128]), one PE
matmul (68-state x 128 sentences), one DVE elementwise multiply.
"""

import sys
import numpy as np

for _p in ("/opt/trn_rl_repo",):
    if _p not in sys.path:
        sys.path.insert(0, _p)

import concourse.bass as bass
import concourse.bacc as bacc
import concourse.tile as tile
from concourse import mybir
from concourse.masks import make_identity
from concourse.bass_utils import run_bass_kernel_spmd

AF = mybir.ActivationFunctionType
OP = mybir.AluOpType

B, T, K = 1024, 512, 32
NCORES = 8
BS = B // NCORES          # 128 sentences per core
S = T + 1                 # 513 time slots (incl. virtual terminal slot)
NCH = 68                  # aug channels / state rows
NST = 98                  # matmul output rows (96/97 = colsums)
CH_T = 9                  # chunks
CL = 57                   # slots per chunk (9*57 = 513)
LNORM = 8                 # renormalize every 8 steps
START_TAG, END_TAG = 30, 31
NEG = -1.0e30
BIG = 1.0e30
F32 = mybir.dt.float32
I32 = mybir.dt.int32


def _emit_kernel(ctx, tc, feats_d, tags_d, mask_d, trans_d, out_d):
    nc = tc.nc

    singles = ctx.enter_context(tc.tile_pool(name="singles", bufs=1))
    aug_pool = ctx.enter_context(tc.tile_pool(name="aug", bufs=2))
    ef_pool = ctx.enter_context(tc.tile_pool(name="ef", bufs=2))
    fm_pool = ctx.enter_context(tc.tile_pool(name="fm", bufs=2))
    u_pool = ctx.enter_context(tc.tile_pool(name="u", bufs=3))
    r_pool = ctx.enter_context(tc.tile_pool(name="r", bufs=2))
    eft_psum = ctx.enter_context(tc.tile_pool(name="eftp", bufs=2, space="PSUM"))
    q_psum = ctx.enter_context(tc.tile_pool(name="qp", bufs=2, space="PSUM"))
    rb_psum = ctx.enter_context(tc.tile_pool(name="rbp", bufs=2, space="PSUM"))
    sm_psum = ctx.enter_context(tc.tile_pool(name="smp", bufs=1, space="PSUM"))

    # ---------------- constants & small precomputes ----------------
    ident = singles.tile([128, 128], F32)
    make_identity(nc, ident[:])

    mask_sb = singles.tile([BS, T], F32)
    nc.sync.dma_start(out=mask_sb[:], in_=mask_d[:, :])
    tags_sb = singles.tile([BS, T], F32)
    nc.sync.dma_start(out=tags_sb[:], in_=tags_d[:, :])

    tT = singles.tile([K, K], F32)
    nc.sync.dma_start(out=tT[:], in_=trans_d.ap().rearrange("a b -> b a"))

    # per-partition index helpers
    io68 = singles.tile([NCH, 1], I32)
    nc.gpsimd.iota(io68[:], pattern=[[0, 1]], base=0, channel_multiplier=1)
    shr1 = singles.tile([NCH, 1], I32)
    nc.vector.tensor_scalar(out=shr1[:], in0=io68[:], scalar1=1, scalar2=None,
                            op0=OP.arith_shift_right)

    # lhsT[p, i] = Eb[i, p] : stationary matrix for the per-step matmul
    lhsT = singles.tile([NCH, NST], F32)
    nc.gpsimd.memset(lhsT[:], 0.0)
    nc.scalar.activation(lhsT[0:32, 0:32], tT[:], AF.Exp)
    nc.scalar.activation(lhsT[32:64, 32:64], tT[:], AF.Exp)
    nc.scalar.activation(lhsT[0:32, 64:65], tT[:, 31:32], AF.Exp)  # ucap col
    nc.gpsimd.memset(lhsT[64:66, 65:66], 1.0)   # uhold <- ucap, uhold
    nc.gpsimd.memset(lhsT[32:64, 66:67], 1.0)   # gcap <- g-real (plain pickup)
    nc.vector.tensor_scalar(out=lhsT[:, 67:68], in0=io68[:], scalar1=66,
                            scalar2=None, op0=OP.is_ge)  # ghold <- gcap, ghold
    scr_a = singles.tile([NCH, 1], F32)
    scr_b = singles.tile([NCH, 1], F32)
    nc.vector.tensor_scalar(out=scr_a[:], in0=io68[:], scalar1=32, scalar2=None,
                            op0=OP.is_lt)
    nc.vector.tensor_scalar(out=scr_b[:], in0=shr1[:], scalar1=32, scalar2=None,
                            op0=OP.is_equal)
    nc.vector.tensor_add(lhsT[:, 96:97], scr_a[:], scr_b[:])      # colsum u
    nc.vector.tensor_scalar(out=lhsT[:, 97:98], in0=lhsT[:, 96:97],
                            scalar1=-1.0, scalar2=1.0, op0=OP.mult, op1=OP.add)

    # sel2: maps r2 = (1/c_u, 1/c_g) onto the 68 state rows
    io2 = singles.tile([2, NCH], I32)
    nc.gpsimd.iota(io2[:], pattern=[[0, NCH]], base=0, channel_multiplier=1)
    sel2 = singles.tile([2, NCH], F32)
    nc.vector.tensor_scalar(out=sel2[:, 0:32], in0=io2[:, 0:32], scalar1=0,
                            scalar2=None, op0=OP.is_equal)
    nc.vector.tensor_scalar(out=sel2[:, 32:64], in0=io2[:, 32:64], scalar1=1,
                            scalar2=None, op0=OP.is_equal)
    nc.vector.tensor_scalar(out=sel2[:, 64:66], in0=io2[:, 64:66], scalar1=0,
                            scalar2=None, op0=OP.is_equal)
    nc.vector.tensor_scalar(out=sel2[:, 66:68], in0=io2[:, 66:68], scalar1=1,
                            scalar2=None, op0=OP.is_equal)

    # capsel: col 0 -> capsum_g rows {63, 66, 67}; col 1 -> capsum_u rows {64, 65}
    capsel = singles.tile([NCH, 2], F32)
    scr_c = singles.tile([NCH, 1], F32)
    nc.vector.tensor_scalar(out=scr_c[:], in0=io68[:], scalar1=63, scalar2=None,
                            op0=OP.is_equal)
    nc.vector.tensor_scalar(out=capsel[:, 0:1], in0=io68[:], scalar1=66,
                            scalar2=None, op0=OP.is_ge)
    nc.vector.tensor_add(capsel[:, 0:1], capsel[:, 0:1], scr_c[:])
    nc.vector.tensor_scalar(out=capsel[:, 1:2], in0=shr1[:], scalar1=32,
                            scalar2=None, op0=OP.is_equal)

    iota32 = singles.tile([128, K], F32)
    nc.gpsimd.iota(iota32[:], pattern=[[1, K]], base=0, channel_multiplier=0,
                   allow_small_or_imprecise_dtypes=True)

    # u0: rows 30 (u START) and 62 (g START) = 1
    io68b = singles.tile([NCH, BS], I32)
    nc.gpsimd.iota(io68b[:], pattern=[[0, BS]], base=0, channel_multiplier=1)
    u0i = singles.tile([NCH, BS], I32)
    nc.vector.tensor_scalar(out=u0i[:], in0=io68b[:], scalar1=31, scalar2=None,
                            op0=OP.bitwise_and)
    u0 = singles.tile([NCH, BS], F32)
    nc.vector.tensor_scalar(out=u0[:], in0=u0i[:], scalar1=30, scalar2=None,
                            op0=OP.is_equal)

    # maskbias[b,t] = (mask-1)*1e30  (0 alive / -1e30 dead)
    maskbias = singles.tile([BS, T], F32)
    nc.vector.tensor_scalar(out=maskbias[:], in0=mask_sb[:], scalar1=1.0,
                            scalar2=BIG, op0=OP.subtract, op1=OP.mult)

    # m_ext = [mask, 0]
    m_ext = singles.tile([BS, S], F32)
    nc.vector.tensor_copy(m_ext[:, 0:T], mask_sb[:])
    nc.gpsimd.memset(m_ext[:, T:S], 0.0)

    # tags2 = [tags0, tags[j] + 100*(1-mask[j-1]) ..., 31 + 100*(1-mask[T-1])]
    tags2 = singles.tile([BS, S], F32)
    nc.vector.tensor_copy(tags2[:, 0:1], tags_sb[:, 0:1])
    nc.vector.scalar_tensor_tensor(out=tags2[:, 1:T], in0=mask_sb[:, 0:T - 1],
                                   scalar=-100.0, in1=tags_sb[:, 1:T],
                                   op0=OP.mult, op1=OP.add)
    nc.vector.tensor_scalar(out=tags2[:, 1:T], in0=tags2[:, 1:T], scalar1=100.0,
                            scalar2=None, op0=OP.add)
    nc.vector.tensor_scalar(out=tags2[:, T:S], in0=mask_sb[:, T - 1:T],
                            scalar1=-100.0, scalar2=131.0, op0=OP.mult, op1=OP.add)

    # capln_u[j]: 0 where j==len else -1e30   (fires at step s=j+1 == len+1)
    capu = singles.tile([BS, S], F32)
    nc.gpsimd.memset(capu[:, 0:1], NEG)
    nc.vector.tensor_sub(capu[:, 1:T], mask_sb[:, 0:T - 1], mask_sb[:, 1:T])
    nc.vector.tensor_copy(capu[:, T:S], mask_sb[:, T - 1:T])
    nc.vector.tensor_scalar(out=capu[:, 1:S], in0=capu[:, 1:S], scalar1=1.0,
                            scalar2=BIG, op0=OP.subtract, op1=OP.mult)

    # capln_g[j]: 0 where j==len+1 else -1e30
    capg = singles.tile([BS, S], F32)
    nc.gpsimd.memset(capg[:, 0:2], NEG)
    nc.vector.tensor_sub(capg[:, 2:S], mask_sb[:, 0:T - 1], mask_sb[:, 1:T])
    nc.vector.tensor_scalar(out=capg[:, 2:S], in0=capg[:, 2:S], scalar1=1.0,
                            scalar2=BIG, op0=OP.subtract, op1=OP.mult)

    # c_histT[b, 2j:2j+2] = (1/c_u, 1/c_g) at boundary j; cols 128:130 = (capg, capu)
    c_histT = singles.tile([BS, 130], F32)

    # ---------------- main recursion ----------------
    GRP = 4                        # transposed slots per PSUM tile / exp op

    aug_tiles = {}

    def build_chunk(cj):
        j0 = cj * CL
        nf = CL if cj < CH_T - 1 else CL - 1   # last chunk: slot 512 is virtual
        aug = aug_pool.tile([BS, CL, NCH], F32, tag="aug")
        nc.sync.dma_start(out=aug[:, 0:nf, 0:32], in_=feats_d[:, j0:j0 + nf, :])
        if cj == CH_T - 1:
            nc.gpsimd.memset(aug[:, CL - 1, 0:32], NEG)
        # cap / hold channels
        nc.vector.tensor_copy(aug[:, :, 64], capu[:, j0:j0 + CL])
        nc.vector.tensor_copy(aug[:, :, 66], capg[:, j0:j0 + CL])
        nc.gpsimd.memset(aug[:, :, 65], 0.0)
        nc.gpsimd.memset(aug[:, :, 67], 0.0)
        # u-real: mask out dead steps
        nc.gpsimd.tensor_tensor(
            aug[:, 0:nf, 0:32], aug[:, 0:nf, 0:32],
            maskbias[:, j0:j0 + nf, None].broadcast_to([BS, nf, K]), OP.add)
        # g-real: noteq(tags2)*(-1e30) + feats*m_ext
        nc.vector.tensor_tensor(
            aug[:, :, 32:64],
            tags2[:, j0:j0 + CL, None].broadcast_to([BS, CL, K]),
            iota32[:, None, :].broadcast_to([BS, CL, K]), OP.not_equal)
        fm = fm_pool.tile([BS, CL, K], F32, tag="fm")
        nc.gpsimd.tensor_tensor(
            fm[:], aug[:, :, 0:32],
            m_ext[:, j0:j0 + CL, None].broadcast_to([BS, CL, K]), OP.mult)
        nc.vector.scalar_tensor_tensor(
            out=aug[:, :, 32:64], in0=aug[:, :, 32:64], scalar=NEG, in1=fm[:],
            op0=OP.mult, op1=OP.add)
        aug_tiles[cj] = aug

    u_prev = u0
    q_prev = None
    for g in range((S + GRP - 1) // GRP):
        slots = list(range(GRP * g, min(GRP * g + GRP, S)))
        ns = len(slots)
        # transpose raw aug columns into one wide PSUM tile, exp -> SBUF
        for j in slots:
            if (j // CL) not in aug_tiles:
                build_chunk(j // CL)
        wide = eft_psum.tile([NCH, GRP * BS], F32, tag="wide")
        for i, j in enumerate(slots):
            nc.tensor.transpose(wide[:, i * BS:(i + 1) * BS],
                                aug_tiles[j // CL][:, j % CL, :], ident[:])
        efsb = ef_pool.tile([NCH, GRP * BS], F32, tag="efsb")
        nc.scalar.activation(efsb[:, 0:ns * BS], wide[:, 0:ns * BS], AF.Exp)
        for i, j in enumerate(slots):
            step = j + 1
            q = q_psum.tile([NST, BS], F32)
            nc.tensor.matmul(q[:], lhsT[:], u_prev[:], start=True, stop=True)
            u = u_pool.tile([NCH, BS], F32)
            nc.vector.tensor_mul(u[:], efsb[:, i * BS:(i + 1) * BS], q[0:NCH, :])
            if step % LNORM == 0 and step <= T:
                bidx = step // LNORM - 1
                r2 = r_pool.tile([2, BS], F32)
                nc.vector.reciprocal(r2[:], q_prev[96:98, :])
                rt = sm_psum.tile([BS, 2], F32, tag="sm")
                nc.tensor.transpose(rt[:], r2[:], ident[0:2, 0:2])
                nc.vector.tensor_copy(c_histT[:, 2 * bidx:2 * bidx + 2], rt[:])
                rb = rb_psum.tile([NCH, BS], F32)
                nc.tensor.matmul(rb[:], sel2[:], r2[:], start=True, stop=True)
                nc.vector.tensor_mul(u[:], u[:], rb[:])
            u_prev, q_prev = u, q

    # ---------------- epilogue ----------------
    capsq = sm_psum.tile([2, BS], F32, tag="sm2")
    nc.tensor.matmul(capsq[:], capsel[:], u_prev[:], start=True, stop=True)
    csb = singles.tile([2, BS], F32)
    nc.vector.tensor_copy(csb[:], capsq[:])
    ct = sm_psum.tile([BS, 2], F32, tag="sm")
    nc.tensor.transpose(ct[:], csb[:], ident[0:2, 0:2])
    nc.vector.tensor_copy(c_histT[:, 128:130], ct[:])

    lnh = singles.tile([BS, 130], F32)
    nc.scalar.activation(lnh[:], c_histT[:], AF.Ln)
    pairs = lnh[:].rearrange("p (a two) -> p a two", two=2)
    diff = singles.tile([BS, 65], F32)
    nc.vector.tensor_sub(diff[:], pairs[:, :, 1], pairs[:, :, 0])
    zd = singles.tile([BS, 1], F32)
    nc.vector.tensor_reduce(zd[:], diff[:], axis=mybir.AxisListType.X, op=OP.add)
    nc.sync.dma_start(out=out_d.ap().rearrange("(a b) -> a b", b=1), in_=zd[:])


def build_module():
    nc = bacc.Bacc("TRN2", target_bir_lowering=False, debug=False)
    feats_d = nc.dram_tensor("feats", [BS, T, K], F32, kind="ExternalInput")
    tags_d = nc.dram_tensor("tags", [BS, T], F32, kind="ExternalInput")
    mask_d = nc.dram_tensor("mask", [BS, T], F32, kind="ExternalInput")
    trans_d = nc.dram_tensor("trans", [K, K], F32, kind="ExternalInput")
    out_d = nc.dram_tensor("out", [BS], F32, kind="ExternalOutput")
    from contextlib import ExitStack
    with tile.TileContext(nc) as tc:
        with ExitStack() as ctx:
            _emit_kernel(ctx, tc, feats_d, tags_d, mask_d, trans_d, out_d)
    nc.compile()
    return nc


_NC_CACHE = None


def _shard_inputs(feats, transitions, tags, sentence_masks):
    feats = np.asarray(feats, dtype=np.float32)
    trans = np.ascontiguousarray(np.asarray(transitions, dtype=np.float32))
    tags = np.asarray(tags).astype(np.float32)
    mask = np.asarray(sentence_masks).astype(np.float32)
    in_maps = []
    for c in range(NCORES):
        sl = slice(c * BS, (c + 1) * BS)
        in_maps.append({
            "feats": np.ascontiguousarray(feats[sl]),
            "tags": np.ascontiguousarray(tags[sl]),
            "mask": np.ascontiguousarray(mask[sl]),
            "trans": trans,
        })
    return in_maps


def kernel(feats, transitions, tags, sentence_masks, _trace=False, _tmpdir=None):
    global _NC_CACHE
    if _NC_CACHE is None:
        _NC_CACHE = build_module()
    nc = _NC_CACHE
    in_maps = _shard_inputs(feats, transitions, tags, sentence_masks)
    res = run_bass_kernel_spmd(nc, in_maps, core_ids=list(range(NCORES)),
                               trace=_trace, tmpdir=_tmpdir)
    out = np.concatenate([res.results[c]["out"] for c in range(NCORES)], axis=0)
    if _trace:
        return out, res
    return out
